# revision 21
# baseline (speedup 1.0000x reference)
"""Trainium2 Bass kernel for the masked ChannelWiseDivergence boundary-KD loss.

Math: for each (b, c) and mask m in {edges, bodies} the reference computes
    CWD(S*m, T*m) = sum_hw pT * (logpT - logpS)
over a softmax across the full HW plane of the mask-multiplied logits.
With t = T*m, s = S*m (masked-out positions contribute logit 0):
    Z_T = sum_m e^T + (N - n) = sum_m (e^T - 1) + N
    Z_S = sum_m (e^S - 1) + N
    CWD = D / Z_T + log Z_S - log Z_T,   D = sum_m e^T (T - S)
Morphology: bodies = erode(m_c), edges = dilate(m_c) - erode(m_c), where
dilate/erode use the 5-point cross. With s5 = #(cross neighborhood == c),
erode = (s5 >= 5), dilate = (s5 >= 1).  edge sums = dilate sums - body sums.
Class 0 is zeroed in the reference and contributes exactly 0.

Sharding: 8 cores = 4 batches x 2 half-planes (rows 0:256 / 256:512), each
core computing per-class partial sums for classes 1..13; final log/divide
math is done on host in float64.

On-chip pipeline per class (engines balanced so VectorE is the bound):
  - DMA S,T half-planes -> exp(S),exp(T) on ScalarE (one bf16 call)
  - one-hot mask via tensor_scalar(is_equal) on VectorE (bf16 gt copy, 4x)
  - 5-point cross sum s5 via TensorE matmuls (tridiagonal + shifted identity
    + cross-block/halo single-element matrices), PSUM-accumulated
  - s5 -> SBUF bf16 on ScalarE; e^x - 1 / T - S / eT*(T-S) on VectorE
  - fused threshold+multiply+accumulate via scalar_tensor_tensor on VectorE:
      accum = sum((s5 > theta) * field),  theta in {0.5, 4.5}
"""

import numpy as np

import concourse.bass as bass
import concourse.bacc as bacc
import concourse.tile as tile
from concourse import mybir
from concourse.bass_utils import run_bass_kernel_spmd

F32 = mybir.dt.float32
BF16 = mybir.dt.bfloat16
I32 = mybir.dt.int32

B, C, H, W = 4, 14, 512, 512
NCLS = C - 1          # classes 1..13
ROWS = 256            # rows per core (half plane)
NBLK = ROWS // 128    # 2 row blocks of 128 partitions
N_HW = H * W
N_CORES = 8
NSUMS = 6             # (dil, body) x (eSm1, eTm1, eTd)
STATS_W = NCLS * NSUMS

_CACHED = {}


def _weight_mats() -> np.ndarray:
    """[128, 6*128] bf16 lhsT weight matrices: tri, I, U, D, Htop, Hbot."""
    P = 128
    wm = np.zeros((P, 6 * P), np.float32)
    tri = wm[:, 0:P]
    for i in range(P):
        for j in (i - 1, i, i + 1):
            if 0 <= j < P:
                tri[j, i] = 1.0
    wm[:, P:2 * P] = np.eye(P)          # I
    wm[0, 2 * P + 127] = 1.0            # U: out[127,blk0] += rhs[0] (blk1)
    wm[127, 3 * P + 0] = 1.0            # D: out[0,blk1] += rhs[127] (blk0)
    wm[0, 4 * P + 0] = 1.0              # Htop: out[0,blk0] += halo[0]
    wm[127, 5 * P + 127] = 1.0          # Hbot: out[127,blk1] += halo[127]
    return wm


def build_nc(repeats: int = 1, gt_bf16: bool = True, use_ttr: bool = False) -> bass.Bass:
    nc = bacc.Bacc(None, target_bir_lowering=False)
    # st_in: S and T stacked; per class one DMA loads both.
    st_in = nc.declare_dram_parameter("st_in", [NCLS, 2, ROWS, W], F32,
                                      isOutput=False)
    # gt packed host-side into the exact SBUF layout [128, NBLK+1, W]
    # (slot NBLK: partition 0 = halo row above, 127 = halo row below, rest -1)
    gt_in = nc.declare_dram_parameter("gt_in", [128, NBLK + 1, W], I32,
                                      isOutput=False)
    wm_in = nc.declare_dram_parameter("wm_in", [128, 6 * 128], BF16, isOutput=False)
    stats_out = nc.declare_dram_parameter("stats", [128, STATS_W], F32, isOutput=True)

    with tile.TileContext(nc) as tc:
        with (
            tc.tile_pool(name="persist", bufs=1) as persist,
            tc.tile_pool(name="st", bufs=3) as st_pool,
            tc.tile_pool(name="bf", bufs=3) as bf_pool,
            tc.tile_pool(name="m", bufs=NCLS) as m_pool,
            tc.tile_pool(name="psum", bufs=2, space="PSUM") as psum_pool,
        ):
            # --- persistent tiles ---
            wm = persist.tile([128, 6 * 128], BF16)
            nc.sync.dma_start(out=wm, in_=wm_in[:, :])

            gt_sb = persist.tile([128, NBLK + 1, W], I32)
            nc.sync.dma_start(out=gt_sb, in_=gt_in[:, :, :])
            if gt_bf16:
                # bf16 copy once: per-class is_equal then runs in 4x mode
                gt_bf = persist.tile([128, NBLK + 1, W], BF16)
                nc.vector.tensor_copy(gt_bf, gt_sb)
            else:
                gt_bf = gt_sb

            stats = persist.tile([128, STATS_W], F32)

            W_TRI = wm[:, 0:128]
            W_I = wm[:, 128:256]
            W_U = wm[:, 256:384]
            W_D = wm[:, 384:512]
            W_HT = wm[:, 512:640]
            W_HB = wm[:, 640:768]

            for rep in range(repeats):
              for ci in range(NCLS):
                c = ci + 1
                # --- load S, T for this class: [128, 2(st), NBLK, W] f32 ---
                st_t = st_pool.tile([128, 2, NBLK, W], F32, tag="st")
                nc.sync.dma_start(
                    out=st_t,
                    in_=st_in[ci].rearrange("s (blk p) w -> p s blk w", p=128),
                )

                # --- one-hot mask (incl. halo slot), 4x bf16 ts ---
                m_t = m_pool.tile([128, NBLK + 1, W], BF16, tag="m")
                nc.vector.tensor_scalar(
                    out=m_t, in0=gt_bf, scalar1=float(c) if gt_bf16 else c,
                    scalar2=None, op0=mybir.AluOpType.is_equal,
                )

                # --- 5-point cross sum via TensorE ---
                s5 = psum_pool.tile([128, NBLK, W], F32, tag="s5")
                for blk in range(NBLK):
                    nc.tensor.matmul(s5[:, blk], W_TRI, m_t[:, blk],
                                     start=True, stop=False)
                for blk in range(NBLK):
                    nc.tensor.matmul(s5[:, blk, 1:W], W_I, m_t[:, blk, 0:W - 1],
                                     start=False, stop=False)
                    nc.tensor.matmul(s5[:, blk, 0:W - 1], W_I, m_t[:, blk, 1:W],
                                     start=False, stop=False)
                # cross-block vertical neighbors
                nc.tensor.matmul(s5[:, 0], W_U, m_t[:, 1],
                                 start=False, stop=False)
                nc.tensor.matmul(s5[:, 1], W_D, m_t[:, 0],
                                 start=False, stop=False)
                # halo rows (slot NBLK: partition 0 = row above, 127 = row below)
                nc.tensor.matmul(s5[:, 0], W_HT, m_t[:, NBLK],
                                 start=False, stop=True)
                nc.tensor.matmul(s5[:, 1], W_HB, m_t[:, NBLK],
                                 start=False, stop=True)

                # --- s5 -> SBUF bf16 (ScalarE; values 0..5 exact) ---
                s5_sb = bf_pool.tile([128, NBLK, W], BF16, tag="s5sb")
                nc.scalar.copy(out=s5_sb, in_=s5)

                # --- exp(S), exp(T) -> bf16 (ScalarE, one call) ---
                est = bf_pool.tile([128, 2, NBLK, W], BF16, tag="est")
                nc.scalar.activation(out=est, in_=st_t,
                                     func=mybir.ActivationFunctionType.Exp)

                # --- e^x - 1 (VectorE ts, 4x bf16) ---
                estm1 = bf_pool.tile([128, 2, NBLK, W], BF16, tag="estm1")
                nc.vector.tensor_scalar(
                    out=estm1, in0=est, scalar1=-1.0, scalar2=None,
                    op0=mybir.AluOpType.add,
                )

                # --- dTS = T - S (f32 -> bf16), eTd = e^T * dTS (VectorE) ---
                dts = bf_pool.tile([128, NBLK, W], BF16, tag="dts")
                nc.vector.tensor_tensor(
                    out=dts, in0=st_t[:, 1], in1=st_t[:, 0],
                    op=mybir.AluOpType.subtract,
                )
                etd = bf_pool.tile([128, NBLK, W], BF16, tag="etd")
                nc.vector.tensor_tensor(
                    out=etd, in0=est[:, 1], in1=dts,
                    op=mybir.AluOpType.mult,
                )

                # --- fused masked sums: accum = sum((s5 > theta) * field) ---
                scratch = bf_pool.tile([128, NBLK, W], BF16, tag="scr")
                fields = (estm1[:, 0], estm1[:, 1], etd)
                if use_ttr:
                    masks = []
                    for mi, theta in enumerate((0.5, 4.5)):
                        mk = bf_pool.tile([128, NBLK, W], BF16, tag=f"mask{mi}")
                        nc.vector.tensor_scalar(
                            out=mk, in0=s5_sb, scalar1=theta, scalar2=None,
                            op0=mybir.AluOpType.is_gt,
                        )
                        masks.append(mk)
                    for mi in range(2):
                        for fi, field in enumerate(fields):
                            col = ci * NSUMS + mi * 3 + fi
                            nc.vector.tensor_tensor_reduce(
                                out=scratch, in0=masks[mi], in1=field,
                                scale=1.0, scalar=0.0,
                                op0=mybir.AluOpType.mult,
                                op1=mybir.AluOpType.add,
                                accum_out=stats[:, col:col + 1],
                            )
                else:
                    for mi, theta in enumerate((0.5, 4.5)):
                        for fi, field in enumerate(fields):
                            col = ci * NSUMS + mi * 3 + fi
                            nc.vector.scalar_tensor_tensor(
                                out=scratch,
                                in0=s5_sb, scalar=theta, in1=field,
                                op0=mybir.AluOpType.is_gt,
                                op1=mybir.AluOpType.mult,
                                accum_out=stats[:, col:col + 1],
                            )

            nc.sync.dma_start(out=stats_out[:, :], in_=stats)
    nc.compile()
    return nc


def _prep_inputs(preds_S, preds_T, gt_labels):
    """Build per-core input maps."""
    S = np.ascontiguousarray(preds_S, dtype=np.float32)
    T = np.ascontiguousarray(preds_T, dtype=np.float32)
    G = np.ascontiguousarray(gt_labels, dtype=np.int32)[:, 0]  # [B, H, W]
    wm = _weight_mats().astype(np.float32)
    import ml_dtypes
    wm_bf = wm.astype(ml_dtypes.bfloat16)
    in_maps = []
    for k in range(N_CORES):
        b, half = divmod(k, 2)
        r0 = half * ROWS
        # st_in: [NCLS, 2, ROWS, W]
        st = np.stack([S[b, 1:C, r0:r0 + ROWS, :],
                       T[b, 1:C, r0:r0 + ROWS, :]], axis=1)
        # gt packed [128, NBLK+1, W]: slot blk<NBLK = row blk*128+p;
        # slot NBLK: partition 0 = halo above, 127 = halo below, rest -1
        gt = np.full((128, NBLK + 1, W), -1, np.int32)
        gt[:, 0:NBLK, :] = (G[b, r0:r0 + ROWS, :]
                            .reshape(NBLK, 128, W).transpose(1, 0, 2))
        if r0 > 0:
            gt[0, NBLK] = G[b, r0 - 1]
        if r0 + ROWS < H:
            gt[127, NBLK] = G[b, r0 + ROWS]
        in_maps.append({
            "st_in": np.ascontiguousarray(st),
            "gt_in": gt,
            "wm_in": wm_bf,
        })
    return in_maps


def _finalize(stats_list):
    """Host-side final math from per-core [128, STATS_W] partial sums."""
    acc = np.zeros((B, NCLS, NSUMS), np.float64)
    for k in range(N_CORES):
        b = k // 2
        acc[b] += np.asarray(stats_list[k], np.float64).sum(axis=0).reshape(NCLS, NSUMS)
    Ad, Bd, Dd = acc[..., 0], acc[..., 1], acc[..., 2]   # dilate sums
    Ab, Bb, Db = acc[..., 3], acc[..., 4], acc[..., 5]   # body (erode) sums
    Ae, Be, De = Ad - Ab, Bd - Bb, Dd - Db               # edge sums
    N = float(N_HW)

    def term(A, Bs, D):
        ZS = A + N
        ZT = Bs + N
        return D / ZT + np.log(ZS) - np.log(ZT)

    loss_e = 500.0 * term(Ae, Be, De).sum() / C / B
    loss_b = 200.0 * term(Ab, Bb, Db).sum() / C / B
    return (np.float32(loss_e), np.float32(loss_b))


def kernel(preds_S, preds_T, gt_labels):
    if "nc" not in _CACHED:
        _CACHED["nc"] = build_nc()
    nc = _CACHED["nc"]
    in_maps = _prep_inputs(preds_S, preds_T, gt_labels)
    res = run_bass_kernel_spmd(nc, in_maps, list(range(N_CORES)))
    stats_list = [r["stats"] for r in res.results]
    return _finalize(stats_list)


if __name__ == "__main__":
    nc = build_nc()
    print("built nc ok")


# revision 22
# speedup vs baseline: 1.2576x; 1.2576x over previous
"""Trainium2 Bass kernel for the masked ChannelWiseDivergence boundary-KD loss.

Math: for each (b, c) and mask m in {edges, bodies} the reference computes
    CWD(S*m, T*m) = sum_hw pT * (logpT - logpS)
over a softmax across the full HW plane of the mask-multiplied logits.
With t = T*m, s = S*m (masked-out positions contribute logit 0):
    Z_T = sum_m e^T + (N - n) = sum_m (e^T - 1) + N
    Z_S = sum_m (e^S - 1) + N
    CWD = D / Z_T + log Z_S - log Z_T,   D = sum_m e^T (T - S)
Morphology: bodies = erode(m_c), edges = dilate(m_c) - erode(m_c), where
dilate/erode use the 5-point cross. With s5 = #(cross neighborhood == c),
erode = (s5 >= 5), dilate = (s5 >= 1).  edge sums = dilate sums - body sums.
Class 0 is zeroed in the reference and contributes exactly 0.

Sharding: 8 cores = 4 batches x 2 half-planes (rows 0:256 / 256:512), each
core computing per-class partial sums for classes 1..13; final log/divide
math is done on host in float64.

On-chip pipeline per class (engines balanced so VectorE is the bound):
  - DMA S,T half-planes as bf16 (host-converted; halves HBM traffic and
    keeps every VectorE op in a packed 2x/4x mode)
  - exp(S),exp(T) on ScalarE (one bf16 call)
  - one-hot mask via tensor_scalar(is_equal) on VectorE (bf16 gt copy, 4x)
  - 5-point cross sum s5 via TensorE matmuls (tridiagonal + shifted identity
    + cross-block/halo single-element matrices), PSUM-accumulated
  - s5 -> SBUF bf16 on ScalarE; e^x - 1 / T - S / eT*(T-S) on VectorE
  - fused threshold+multiply+accumulate via scalar_tensor_tensor on VectorE:
      accum = sum((s5 > theta) * field),  theta in {0.5, 4.5}
"""

import numpy as np

import concourse.bass as bass
import concourse.bacc as bacc
import concourse.tile as tile
from concourse import mybir
from concourse.bass_utils import run_bass_kernel_spmd

F32 = mybir.dt.float32
BF16 = mybir.dt.bfloat16
I32 = mybir.dt.int32

B, C, H, W = 4, 14, 512, 512
NCLS = C - 1          # classes 1..13
ROWS = 256            # rows per core (half plane)
NBLK = ROWS // 128    # 2 row blocks of 128 partitions
N_HW = H * W
N_CORES = 8
NSUMS = 6             # (dil, body) x (eSm1, eTm1, eTd)
STATS_W = NCLS * NSUMS

_CACHED = {}


def _weight_mats() -> np.ndarray:
    """[128, 6*128] bf16 lhsT weight matrices: tri, I, U, D, Htop, Hbot."""
    P = 128
    wm = np.zeros((P, 6 * P), np.float32)
    tri = wm[:, 0:P]
    for i in range(P):
        for j in (i - 1, i, i + 1):
            if 0 <= j < P:
                tri[j, i] = 1.0
    wm[:, P:2 * P] = np.eye(P)          # I
    wm[0, 2 * P + 127] = 1.0            # U: out[127,blk0] += rhs[0] (blk1)
    wm[127, 3 * P + 0] = 1.0            # D: out[0,blk1] += rhs[127] (blk0)
    wm[0, 4 * P + 0] = 1.0              # Htop: out[0,blk0] += halo[0]
    wm[127, 5 * P + 127] = 1.0          # Hbot: out[127,blk1] += halo[127]
    return wm


def build_nc(repeats: int = 1, gt_bf16: bool = True, use_ttr: bool = False) -> bass.Bass:
    nc = bacc.Bacc(None, target_bir_lowering=False)
    # st_in: S and T stacked; per class one DMA loads both.
    st_in = nc.declare_dram_parameter("st_in", [NCLS, 2, ROWS, W], BF16,
                                      isOutput=False)
    # gt packed host-side into the exact SBUF layout [128, NBLK+1, W]
    # (slot NBLK: partition 0 = halo row above, 127 = halo row below, rest -1)
    gt_in = nc.declare_dram_parameter("gt_in", [128, NBLK + 1, W], I32,
                                      isOutput=False)
    wm_in = nc.declare_dram_parameter("wm_in", [128, 6 * 128], BF16, isOutput=False)
    stats_out = nc.declare_dram_parameter("stats", [128, STATS_W], F32, isOutput=True)

    with tile.TileContext(nc) as tc:
        with (
            tc.tile_pool(name="persist", bufs=1) as persist,
            tc.tile_pool(name="st", bufs=4) as st_pool,
            tc.tile_pool(name="bf", bufs=3) as bf_pool,
            tc.tile_pool(name="m", bufs=NCLS) as m_pool,
            tc.tile_pool(name="psum", bufs=2, space="PSUM") as psum_pool,
        ):
            # --- persistent tiles ---
            wm = persist.tile([128, 6 * 128], BF16)
            nc.sync.dma_start(out=wm, in_=wm_in[:, :])

            gt_sb = persist.tile([128, NBLK + 1, W], I32)
            nc.sync.dma_start(out=gt_sb, in_=gt_in[:, :, :])
            if gt_bf16:
                # bf16 copy once: per-class is_equal then runs in 4x mode
                gt_bf = persist.tile([128, NBLK + 1, W], BF16)
                nc.vector.tensor_copy(gt_bf, gt_sb)
            else:
                gt_bf = gt_sb

            stats = persist.tile([128, STATS_W], F32)

            W_TRI = wm[:, 0:128]
            W_I = wm[:, 128:256]
            W_U = wm[:, 256:384]
            W_D = wm[:, 384:512]
            W_HT = wm[:, 512:640]
            W_HB = wm[:, 640:768]

            for rep in range(repeats):
              for ci in range(NCLS):
                c = ci + 1
                # --- load S, T for this class: [128, 2(st), NBLK, W] f32 ---
                st_t = st_pool.tile([128, 2, NBLK, W], BF16, tag="st")
                nc.sync.dma_start(
                    out=st_t,
                    in_=st_in[ci].rearrange("s (blk p) w -> p s blk w", p=128),
                )

                # --- one-hot mask (incl. halo slot), 4x bf16 ts ---
                m_t = m_pool.tile([128, NBLK + 1, W], BF16, tag="m")
                nc.vector.tensor_scalar(
                    out=m_t, in0=gt_bf, scalar1=float(c) if gt_bf16 else c,
                    scalar2=None, op0=mybir.AluOpType.is_equal,
                )

                # --- 5-point cross sum via TensorE ---
                s5 = psum_pool.tile([128, NBLK, W], F32, tag="s5")
                for blk in range(NBLK):
                    nc.tensor.matmul(s5[:, blk], W_TRI, m_t[:, blk],
                                     start=True, stop=False)
                for blk in range(NBLK):
                    nc.tensor.matmul(s5[:, blk, 1:W], W_I, m_t[:, blk, 0:W - 1],
                                     start=False, stop=False)
                    nc.tensor.matmul(s5[:, blk, 0:W - 1], W_I, m_t[:, blk, 1:W],
                                     start=False, stop=False)
                # cross-block vertical neighbors
                nc.tensor.matmul(s5[:, 0], W_U, m_t[:, 1],
                                 start=False, stop=False)
                nc.tensor.matmul(s5[:, 1], W_D, m_t[:, 0],
                                 start=False, stop=False)
                # halo rows (slot NBLK: partition 0 = row above, 127 = row below)
                nc.tensor.matmul(s5[:, 0], W_HT, m_t[:, NBLK],
                                 start=False, stop=True)
                nc.tensor.matmul(s5[:, 1], W_HB, m_t[:, NBLK],
                                 start=False, stop=True)

                # --- s5 -> SBUF bf16 (ScalarE; values 0..5 exact) ---
                s5_sb = bf_pool.tile([128, NBLK, W], BF16, tag="s5sb")
                nc.scalar.copy(out=s5_sb, in_=s5)

                # --- exp(S), exp(T) -> bf16 (ScalarE, one call) ---
                est = bf_pool.tile([128, 2, NBLK, W], BF16, tag="est")
                nc.scalar.activation(out=est, in_=st_t,
                                     func=mybir.ActivationFunctionType.Exp)

                # --- e^x - 1 (VectorE ts, 4x bf16) ---
                estm1 = bf_pool.tile([128, 2, NBLK, W], BF16, tag="estm1")
                nc.vector.tensor_scalar(
                    out=estm1, in0=est, scalar1=-1.0, scalar2=None,
                    op0=mybir.AluOpType.add,
                )

                # --- dTS = T - S (f32 -> bf16), eTd = e^T * dTS (VectorE) ---
                dts = bf_pool.tile([128, NBLK, W], BF16, tag="dts")
                nc.vector.tensor_tensor(
                    out=dts, in0=st_t[:, 1], in1=st_t[:, 0],
                    op=mybir.AluOpType.subtract,
                )
                etd = bf_pool.tile([128, NBLK, W], BF16, tag="etd")
                nc.vector.tensor_tensor(
                    out=etd, in0=est[:, 1], in1=dts,
                    op=mybir.AluOpType.mult,
                )

                # --- fused masked sums: accum = sum((s5 > theta) * field) ---
                scratch = bf_pool.tile([128, NBLK, W], BF16, tag="scr")
                fields = (estm1[:, 0], estm1[:, 1], etd)
                if use_ttr:
                    masks = []
                    for mi, theta in enumerate((0.5, 4.5)):
                        mk = bf_pool.tile([128, NBLK, W], BF16, tag=f"mask{mi}")
                        nc.vector.tensor_scalar(
                            out=mk, in0=s5_sb, scalar1=theta, scalar2=None,
                            op0=mybir.AluOpType.is_gt,
                        )
                        masks.append(mk)
                    for mi in range(2):
                        for fi, field in enumerate(fields):
                            col = ci * NSUMS + mi * 3 + fi
                            nc.vector.tensor_tensor_reduce(
                                out=scratch, in0=masks[mi], in1=field,
                                scale=1.0, scalar=0.0,
                                op0=mybir.AluOpType.mult,
                                op1=mybir.AluOpType.add,
                                accum_out=stats[:, col:col + 1],
                            )
                else:
                    for mi, theta in enumerate((0.5, 4.5)):
                        for fi, field in enumerate(fields):
                            col = ci * NSUMS + mi * 3 + fi
                            nc.vector.scalar_tensor_tensor(
                                out=scratch,
                                in0=s5_sb, scalar=theta, in1=field,
                                op0=mybir.AluOpType.is_gt,
                                op1=mybir.AluOpType.mult,
                                accum_out=stats[:, col:col + 1],
                            )

            nc.sync.dma_start(out=stats_out[:, :], in_=stats)
    nc.compile()
    return nc


def _prep_inputs(preds_S, preds_T, gt_labels):
    """Build per-core input maps."""
    import ml_dtypes
    S = np.ascontiguousarray(preds_S, dtype=np.float32)
    T = np.ascontiguousarray(preds_T, dtype=np.float32)
    G = np.ascontiguousarray(gt_labels, dtype=np.int32)[:, 0]  # [B, H, W]
    wm = _weight_mats().astype(np.float32)
    wm_bf = wm.astype(ml_dtypes.bfloat16)
    in_maps = []
    for k in range(N_CORES):
        b, half = divmod(k, 2)
        r0 = half * ROWS
        # st_in: [NCLS, 2, ROWS, W]
        st = np.stack([S[b, 1:C, r0:r0 + ROWS, :],
                       T[b, 1:C, r0:r0 + ROWS, :]], axis=1)
        st = st.astype(ml_dtypes.bfloat16)
        # gt packed [128, NBLK+1, W]: slot blk<NBLK = row blk*128+p;
        # slot NBLK: partition 0 = halo above, 127 = halo below, rest -1
        gt = np.full((128, NBLK + 1, W), -1, np.int32)
        gt[:, 0:NBLK, :] = (G[b, r0:r0 + ROWS, :]
                            .reshape(NBLK, 128, W).transpose(1, 0, 2))
        if r0 > 0:
            gt[0, NBLK] = G[b, r0 - 1]
        if r0 + ROWS < H:
            gt[127, NBLK] = G[b, r0 + ROWS]
        in_maps.append({
            "st_in": np.ascontiguousarray(st),
            "gt_in": gt,
            "wm_in": wm_bf,
        })
    return in_maps


def _finalize(stats_list):
    """Host-side final math from per-core [128, STATS_W] partial sums."""
    acc = np.zeros((B, NCLS, NSUMS), np.float64)
    for k in range(N_CORES):
        b = k // 2
        acc[b] += np.asarray(stats_list[k], np.float64).sum(axis=0).reshape(NCLS, NSUMS)
    Ad, Bd, Dd = acc[..., 0], acc[..., 1], acc[..., 2]   # dilate sums
    Ab, Bb, Db = acc[..., 3], acc[..., 4], acc[..., 5]   # body (erode) sums
    Ae, Be, De = Ad - Ab, Bd - Bb, Dd - Db               # edge sums
    N = float(N_HW)

    def term(A, Bs, D):
        ZS = A + N
        ZT = Bs + N
        return D / ZT + np.log(ZS) - np.log(ZT)

    loss_e = 500.0 * term(Ae, Be, De).sum() / C / B
    loss_b = 200.0 * term(Ab, Bb, Db).sum() / C / B
    return (np.float32(loss_e), np.float32(loss_b))


def kernel(preds_S, preds_T, gt_labels):
    if "nc" not in _CACHED:
        _CACHED["nc"] = build_nc()
    nc = _CACHED["nc"]
    in_maps = _prep_inputs(preds_S, preds_T, gt_labels)
    res = run_bass_kernel_spmd(nc, in_maps, list(range(N_CORES)))
    stats_list = [r["stats"] for r in res.results]
    return _finalize(stats_list)


if __name__ == "__main__":
    nc = build_nc()
    print("built nc ok")
